# revision 12
# baseline (speedup 1.0000x reference)
"""Trainium2 Bass kernel for nn_AvgPoolingModel (embedding avg-pool + tiny MLP).

Model:  emb = table[batch]           # [B, L, 300] gather
        pooled = emb.sum(1) / lens   # [B, 300]
        h1 = relu(pooled @ W1.T + b1)
        h2 = relu(h1 @ W2.T + b2)
        y  = (h2 @ W3.T + b3)[:, 0]  # [B]

Sharding: data-parallel over B across 8 cores (512 rows/core); embedding
table + MLP weights replicated per core.

Gather strategy (v2): the SWDGE ucode gather (`gpsimd.dma_gather`,
InstDMAGatherAnt) amortizes the ~1us fixed Pool-DMA instruction overhead
over thousands of tokens per call, vs one 128-descriptor indirect_dma_start
per sequence position (~800 calls -> ~1.2ms issue-bound).  dma_gather
indices are int16, so each call's index window must span <= 32768 table
rows: indices are pre-sorted per batch row on the host, and calls cover
narrow per-quantile column chunks of the sorted matrix with a host-computed
shared base row (data-dependent, same for all cores -> SPMD-safe).  The
table is uploaded as f32 padded to 320 elems (1280B rows, the 256B-aligned
stride the ucode requires); fp16 storage was tried and rejected — table
quantization noise pushes small-|y| outputs past the 2e-2 rel-err gate.

Per call: tokens t = c*128 + p map to (row p of the 128-row tile, sorted
column c); the gather lands token t at partition t%128, so row p's values
arrive along partition p's free dim ([128, w, 320] f32).  A contiguous
in-place pairwise tree-add collapses the w columns into a [128, 320] f32
accumulator (a strided reduce_sum view runs 2-4x below DVE stream rate).
Gathers rotate over 4 SWDGE queues (parallel Q7 desc-gen cpu pairs +
rings), ~3x single-queue gather throughput, and are split into <=5-column
sub-calls so each call's single_packet-coalesced SDMA stream stays at the
proven-safe 819KB (a >~1MB coalesced packet is FATAL to the device:
NRT_EXEC_UNIT_UNRECOVERABLE).  The 3-layer MLP epilogue (PE transposes +
matmuls, ones-row bias trick) is unchanged from v1.

Measured (1-core gather-only slope, 131MB): 912us single-queue
per-desc-packets -> 313us on 4 queues -> 253us with 819KB coalesced
packets.  Full kernel: 838us (strided reduce) -> 371us (tree-add) ->
~320us (coalesced sub-calls), vs the v1 indirect-DMA baseline's
1.05-1.5ms.
"""

import numpy as np

import concourse.bass as bass
import concourse.mybir as mybir
from concourse.tile import TileContext

VOCAB, EMB = 100000, 300
EPAD = 320                     # f32 row padded to 1280B (256B multiple)
B, L = 4096, 200
H1, H2 = 150, 150
NCORES = 8
BC = B // NCORES               # rows per core
P = 128
NT = BC // P                   # row-tiles per core
SPAN_CAP = 32000               # greedy chunk span budget (int16 max 32767)
WMAX = 16                      # max sorted-columns per gather call
RBUFS = 7                      # ring depth for gather tiles
SUBW = 5                       # columns per gather sub-call: 5*128*1280B =
                               # 819KB single-packet, the proven-safe size

F32 = mybir.dt.float32
F16 = mybir.dt.float16
I32 = mybir.dt.int32
I16 = mybir.dt.int16


def plan_chunks(srt):
    """Greedy column chunks of the row-sorted index matrix.

    srt: [B, L] row-sorted indices (ALL rows -> chunk plan + bases are
    identical for every core, keeping the program SPMD).
    Returns list of (col0, width, base, rows): call gathers columns
    [col0, col0+width) of each row with table window [base, base+rows).
    """
    colmin = srt.min(axis=0).astype(np.int64)
    colmax = srt.max(axis=0).astype(np.int64)
    chunks = []
    k = 0
    while k < L:
        w = 1
        while (
            k + w < L
            and w < WMAX
            and colmax[k + w] - colmin[k] <= SPAN_CAP
        ):
            w += 1
        base = int(colmin[k])
        span = int(colmax[k + w - 1]) - base
        assert span <= 32767, (k, w, span)
        chunks.append((k, w, base, span + 1))
        k += w
    return chunks


NQ = 4                         # SWDGE queues (desc-gen cpu pairs) used


def build_nc(chunks, repeat=None):
    """Build the per-core Bass kernel for a given chunk plan.

    repeat=None: the real kernel.  repeat=R: the gather+reduce+MLP body is
    python-unrolled R times (identical work per rep), used only for
    wall-clock slope timing.  (A hardware For_i would force the staggered
    5-sem SWDGE pool, which is incompatible with multi-queue: Tile rotates
    DMASW sems mod 8 per Pool-DMA instruction, so queue = counter % NQ is
    consistent only when NQ divides 8.)

    Gathers rotate over NQ SWDGE queues: each queue's desc-gen runs on its
    own Q7 cpu pair and feeds its own descriptor ring, ~3x the single-queue
    gather throughput (measured 912us -> 313us per 131MB iteration).
    """
    from concourse import bacc
    from concourse.masks import make_identity

    nc = bacc.Bacc("TRN2", target_bir_lowering=False, debug=False,
                   num_swdge_queues=NQ)

    ncols = 8 * L * NT  # sum over (tile, chunk) of 8*w = 8*L per tile
    idx_d = nc.dram_tensor("idx", [P, ncols], I16, kind="ExternalInput")
    emb_d = nc.dram_tensor("emb32", [VOCAB, EPAD], F32, kind="ExternalInput")
    recip_d = nc.dram_tensor("recip", [BC], F32, kind="ExternalInput")
    w1t_d = nc.dram_tensor("w1t", [EMB, H1], F32, kind="ExternalInput")
    b1_d = nc.dram_tensor("b1", [H1], F32, kind="ExternalInput")
    w2t_d = nc.dram_tensor("w2t", [H1, H2], F32, kind="ExternalInput")
    b2_d = nc.dram_tensor("b2", [H2], F32, kind="ExternalInput")
    w3t_d = nc.dram_tensor("w3t", [H2, 1], F32, kind="ExternalInput")
    b3_d = nc.dram_tensor("b3", [1], F32, kind="ExternalInput")
    y_d = nc.dram_tensor("y", [BC], F32, kind="ExternalOutput")

    with TileContext(nc) as tc:
        with (
            tc.tile_pool(name="const", bufs=1) as cpool,
            tc.tile_pool(name="gat", bufs=1) as gpool,
            tc.tile_pool(name="ring", bufs=2) as rpool,
            tc.tile_pool(name="work", bufs=2) as wpool,
            tc.tile_pool(name="psum", bufs=1, space="PSUM") as ppool,
            tc.tile_pool(name="psum2", bufs=2, space="PSUM") as ppool2,
        ):
            # ---- one-time constants -------------------------------------
            identity = cpool.tile([P, P], F32)
            make_identity(nc, identity[:])
            ones_row = cpool.tile([1, P], F32)
            nc.vector.memset(ones_row[:], 1.0)

            w1t_sb = cpool.tile([100, 3 * H1], F32)   # 3 K-chunks of W1.T
            for c in range(3):
                nc.sync.dma_start(
                    out=w1t_sb[:, c * H1:(c + 1) * H1],
                    in_=w1t_d[c * 100:(c + 1) * 100, :],
                )
            w2t_sb = cpool.tile([75, 2 * H2], F32)    # 2 K-chunks of W2.T
            for c in range(2):
                nc.sync.dma_start(
                    out=w2t_sb[:, c * H2:(c + 1) * H2],
                    in_=w2t_d[c * 75:(c + 1) * 75, :],
                )
            w3t_sb = cpool.tile([75, 2], F32)         # 2 K-chunks of W3.T
            for c in range(2):
                nc.sync.dma_start(
                    out=w3t_sb[:, c:c + 1], in_=w3t_d[c * 75:(c + 1) * 75, :]
                )
            b1_sb = cpool.tile([1, H1], F32)
            nc.sync.dma_start(out=b1_sb[:], in_=b1_d[None, :])
            b2_sb = cpool.tile([1, H2], F32)
            nc.sync.dma_start(out=b2_sb[:], in_=b2_d[None, :])
            b3_sb = cpool.tile([1, 1], F32)
            nc.sync.dma_start(out=b3_sb[:], in_=b3_d[None, :])

            recip_sb = cpool.tile([P, NT], F32)
            nc.sync.dma_start(
                out=recip_sb[:], in_=recip_d.ap().rearrange("(t p) -> p t", p=P)
            )
            out_sb = cpool.tile([P, NT], F32)

            idx_sb = cpool.tile([P, ncols], I16)
            nc.sync.dma_start(out=idx_sb[:], in_=idx_d[:])

            accs = [
                gpool.tile([P, EPAD], F32, tag=f"acc{t}", name=f"acc{t}")
                for t in range(NT)
            ]

            # ---- gather stream (SWDGE ucode gather) + DVE reduce --------
            qctr = [0]

            def gather_and_reduce(it=""):
                off = 0
                for t in range(NT):
                    for j, (k, w, base, rows) in enumerate(chunks):
                        g = rpool.tile(
                            [P, WMAX * EPAD], F32, tag="g", bufs=RBUFS,
                            name=f"g{t}_{j}{it}",
                        )
                        # Sub-calls of <=SUBW columns so each call's
                        # single_packet stream stays <=819KB: coalescing a
                        # call into one SDMA packet is ~20% faster than
                        # per-descriptor packets, but a >~1MB packet kills
                        # the device (NRT_EXEC_UNIT_UNRECOVERABLE).
                        for a0 in range(0, w, SUBW):
                            wa = min(SUBW, w - a0)
                            out3 = g[:, a0 * EPAD : (a0 + wa) * EPAD].rearrange(
                                "p (c e) -> p c e", e=EPAD
                            )
                            nc.gpsimd.dma_gather(
                                out3,
                                emb_d[base : base + rows, :],
                                idx_sb[:, off + 8 * a0 : off + 8 * (a0 + wa)],
                                P * wa,
                                P * wa,
                                EPAD,
                                queue_num=qctr[0] % NQ,
                                single_packet=True,
                            )
                            qctr[0] += 1
                        off += 8 * w
                        # Contiguous in-place pairwise-halving tree sum over
                        # the w gathered columns.  A strided reduce_sum view
                        # ("p (c e) -> p e c", inner stride 1280B) runs 2-4x
                        # below DVE stream rate; stride-1 adds do not.
                        n = w
                        while n > 1:
                            h = n // 2
                            nc.vector.tensor_add(
                                out=g[:, : h * EPAD],
                                in0=g[:, : h * EPAD],
                                in1=g[:, h * EPAD : 2 * h * EPAD],
                            )
                            if n % 2:
                                nc.vector.tensor_add(
                                    out=g[:, :EPAD],
                                    in0=g[:, :EPAD],
                                    in1=g[:, (n - 1) * EPAD : n * EPAD],
                                )
                            n = h
                        if j == 0:
                            nc.vector.tensor_copy(
                                out=accs[t][:], in_=g[:, :EPAD]
                            )
                        else:
                            nc.vector.tensor_add(
                                out=accs[t][:],
                                in0=accs[t][:],
                                in1=g[:, :EPAD],
                            )

            # ---- per-row-tile epilogue: scale + MLP ---------------------
            def epilogue(it=""):
                for t in range(NT):
                    acc = accs[t]
                    scaled = wpool.tile([P, EMB], F32, tag="scaled",
                                        name=f"scaled{t}{it}")
                    nc.vector.tensor_scalar_mul(
                        scaled[:], acc[:, :EMB], recip_sb[:, t:t + 1]
                    )

                    pooledT = wpool.tile([100, 3 * P], F32, tag="pooledT",
                                         name=f"pooledT{t}{it}")
                    for c in range(3):
                        tp_ps = ppool2.tile([100, P], F32, tag="tps",
                                            name=f"tp{t}_{c}{it}")
                        nc.tensor.transpose(
                            out=tp_ps[:], in_=scaled[:, c * 100:(c + 1) * 100],
                            identity=identity[:],
                        )
                        nc.scalar.copy(pooledT[:, c * P:(c + 1) * P], tp_ps[:])

                    h1_ps = ppool.tile([P, H1], F32, tag="h1", name=f"h1ps{t}{it}")
                    for c in range(3):
                        nc.tensor.matmul(
                            out=h1_ps[:],
                            lhsT=pooledT[:, c * P:(c + 1) * P],
                            rhs=w1t_sb[:, c * H1:(c + 1) * H1],
                            start=(c == 0), stop=False,
                        )
                    nc.tensor.matmul(
                        out=h1_ps[:], lhsT=ones_row[:], rhs=b1_sb[:],
                        start=False, stop=True,
                    )
                    h1_sb = wpool.tile([P, H1], F32, tag="h1sb", name=f"h1sb{t}{it}")
                    nc.scalar.activation(
                        h1_sb[:], h1_ps[:], mybir.ActivationFunctionType.Relu
                    )

                    h1t = wpool.tile([75, 2 * P], F32, tag="h1t", name=f"h1t{t}{it}")
                    for c in range(2):
                        t1_ps = ppool2.tile([75, P], F32, tag="tps",
                                            name=f"t1{t}_{c}{it}")
                        nc.tensor.transpose(
                            out=t1_ps[:], in_=h1_sb[:, c * 75:(c + 1) * 75],
                            identity=identity[:],
                        )
                        nc.scalar.copy(h1t[:, c * P:(c + 1) * P], t1_ps[:])

                    h2_ps = ppool.tile([P, H2], F32, tag="h2", name=f"h2ps{t}{it}")
                    for c in range(2):
                        nc.tensor.matmul(
                            out=h2_ps[:],
                            lhsT=h1t[:, c * P:(c + 1) * P],
                            rhs=w2t_sb[:, c * H2:(c + 1) * H2],
                            start=(c == 0), stop=False,
                        )
                    nc.tensor.matmul(
                        out=h2_ps[:], lhsT=ones_row[:], rhs=b2_sb[:],
                        start=False, stop=True,
                    )
                    h2_sb = wpool.tile([P, H2], F32, tag="h2sb", name=f"h2sb{t}{it}")
                    nc.scalar.activation(
                        h2_sb[:], h2_ps[:], mybir.ActivationFunctionType.Relu
                    )

                    h2t = wpool.tile([75, 2 * P], F32, tag="h2t", name=f"h2t{t}{it}")
                    for c in range(2):
                        t2_ps = ppool2.tile([75, P], F32, tag="tps",
                                            name=f"t2{t}_{c}{it}")
                        nc.tensor.transpose(
                            out=t2_ps[:], in_=h2_sb[:, c * 75:(c + 1) * 75],
                            identity=identity[:],
                        )
                        nc.scalar.copy(h2t[:, c * P:(c + 1) * P], t2_ps[:])

                    y_ps = ppool.tile([P, 1], F32, tag="y", name=f"yps{t}{it}")
                    for c in range(2):
                        nc.tensor.matmul(
                            out=y_ps[:],
                            lhsT=h2t[:, c * P:(c + 1) * P],
                            rhs=w3t_sb[:, c:c + 1],
                            start=(c == 0), stop=False,
                        )
                    nc.tensor.matmul(
                        out=y_ps[:], lhsT=ones_row[:], rhs=b3_sb[:],
                        start=False, stop=True,
                    )
                    nc.scalar.copy(out_sb[:, t:t + 1], y_ps[:])

                nc.sync.dma_start(
                    out=y_d.ap().rearrange("(t p) -> p t", p=P), in_=out_sb[:]
                )

            if repeat is None:
                gather_and_reduce()
                epilogue()
            else:
                for rep in range(repeat):
                    gather_and_reduce(f"_r{rep}")
                    epilogue(f"_r{rep}")

    nc.compile()
    return nc


def _pack_idx_core(srt_core, chunks):
    """Build the [128, ncols] int16 index tensor for one core.

    Token t = c*128 + p of call (tile, chunk) reads sorted column
    chunk_col0+c of tile-row p; the ucode consumes indices at
    [t % 16, t // 16] in a [16, n/16] block, replicated x8 to 128
    partitions (each SWDGE Q7 cpu pair reads its own 16-partition stripe).
    """
    cols = []
    for t in range(NT):
        tile_rows = srt_core[t * P:(t + 1) * P]
        for (k, w, base, rows) in chunks:
            loc = (tile_rows[:, k:k + w] - base).astype(np.int16)  # [128, w]
            tok = loc.T.reshape(-1)            # token t=c*128+p -> loc[p, c]
            cols.append(tok.reshape(-1, 16).T)  # [16, 8w]
    idx16 = np.concatenate(cols, axis=1)
    return np.tile(idx16, (8, 1))


def prep_in_maps(batch, lens, emb_table, W1, b1, W2, b2, W3, b3):
    batch = np.asarray(batch, dtype=np.int64)
    srt = np.sort(batch, axis=1).astype(np.int32)
    chunks = plan_chunks(srt)

    lens_f = np.asarray(lens).astype(np.float32)
    recip = (np.float32(1.0) / lens_f).astype(np.float32)

    emb32 = np.zeros((VOCAB, EPAD), dtype=np.float32)
    emb32[:, :EMB] = np.asarray(emb_table, dtype=np.float32)

    common = {
        "emb32": emb32,
        "w1t": np.ascontiguousarray(np.asarray(W1, np.float32).T),
        "b1": np.asarray(b1, np.float32),
        "w2t": np.ascontiguousarray(np.asarray(W2, np.float32).T),
        "b2": np.asarray(b2, np.float32),
        "w3t": np.ascontiguousarray(np.asarray(W3, np.float32).T),
        "b3": np.asarray(b3, np.float32),
    }
    in_maps = []
    for c in range(NCORES):
        sl = slice(c * BC, (c + 1) * BC)
        in_maps.append({
            "idx": _pack_idx_core(srt[sl], chunks),
            "recip": recip[sl],
            **common,
        })
    return in_maps, chunks


_NC_CACHE = {}


def kernel(batch, lens, emb_table, W1, b1, W2, b2, W3, b3):
    from concourse.bass_utils import run_bass_kernel_spmd

    in_maps, chunks = prep_in_maps(
        batch, lens, emb_table, W1, b1, W2, b2, W3, b3
    )
    key = tuple(chunks)
    if key not in _NC_CACHE:
        _NC_CACHE.clear()
        _NC_CACHE[key] = build_nc(chunks)
    nc = _NC_CACHE[key]
    last_err = None
    for _attempt in range(3):
        try:
            res = run_bass_kernel_spmd(nc, in_maps, core_ids=list(range(NCORES)))
            break
        except Exception as e:  # transient axon desync/device-state errors
            last_err = e
            import time as _time

            _time.sleep(5.0)
    else:
        raise last_err
    out = np.concatenate([r["y"] for r in res.results])
    return out.astype(np.float32)
